# revision 53
# baseline (speedup 1.0000x reference)
"""Trainium2 Bass kernel for a dense transformer block (pre-LN, MHA + MLP).

Sharding: 8 cores; core c handles batch b = c // 4, query block qb = c % 4
(512 tokens). Each core recomputes K/V for its batch's full 2048-token
sequence (zero cross-core communication), then runs attention for its
512 query tokens and the MLP on them.

All matmul inputs are bf16 (residual path stays f32); every weight is
loaded once. The host pre-arranges all inputs into the exact on-chip tile
layout ([P, ...] per-partition contiguous rows), split into per-ct
sub-DMAs for DMA-engine parallelism. Phase B1 streams 512-token chunks:
LN1 -> K proj (all 8 head pairs) -> V proj, so lnkv is a rotating 2-buffer
stream. Phase B2 is head-pair-outer attention with AV accumulated across
all 16 kv subchunks in PSUM (ACT-exp-bound at ~96% occupancy, the floor
for this sharding); the two K=64 score matmuls of a head pair go to PE
row groups 0/64 and run concurrently (PE row tiling). LayerNorm rstd uses
exp(-0.5*ln(var+eps)) on ACT; mu/rstd and softmax 1/den broadcasts along
partitions use PE ones-outer-products into PSUM instead of slow gpsimd
partition_broadcast; softmax denominators get one batched DVE reciprocal
per head pair, deferred into the next head pair's score loop so the PE
never waits on it.
"""
import numpy as np

import concourse.bass as bass
import concourse.mybir as mybir
import concourse.tile as tile
from concourse import bacc
from concourse.bass_utils import run_bass_kernel_spmd

P = 128
C = 1024
NCT = C // P          # 8 feature tiles
TKV = 2048            # kv tokens per core (sequence length)
TQ = 512              # query tokens per core
HID = 4096
NHT = HID // P        # 32 hidden tiles
H = 16
HD = 64
NHP = H // 2          # 8 head pairs
NJL = TKV // P        # 16 kv subchunks of 128
CHUNK = 512           # ln1/K/V processing chunk
NCHUNK = TKV // CHUNK # 4
EPS = 1e-5
SCALE = HD ** -0.5

f32 = mybir.dt.float32
f32r = mybir.dt.float32r
bf16 = mybir.dt.bfloat16
Act = mybir.ActivationFunctionType


def build_program(sim_standin=False):
    # CoreSim lacks Gelu; Tanh has identical ACT cost, so the sim variant
    # swaps it in for modeled-time runs.
    gelu_fn = Act.Tanh if sim_standin else Act.Gelu
    nc = bacc.Bacc()

    # DRAM I/O (per core). Host pre-arranges everything into the exact
    # on-chip tile layout, so every DMA is per-partition contiguous
    # (128 descriptors instead of 1024+).
    xkvA = nc.dram_tensor("xkvA", [P, NCHUNK, NCT, CHUNK], bf16,
                          kind="ExternalInput")
    xqbA = nc.dram_tensor("xqbA", [P, NCT, TQ], bf16, kind="ExternalInput")
    xqA = nc.dram_tensor("xqA", [P, NCT, TQ], f32r, kind="ExternalInput")
    wqA = nc.dram_tensor("wqA", [P, NHP, NCT, P], bf16,
                         kind="ExternalInput")
    wkA = nc.dram_tensor("wkA", [P, NHP, NCT, P], bf16,
                         kind="ExternalInput")
    wvA = nc.dram_tensor("wvA", [P, NCT, C], bf16, kind="ExternalInput")
    wpA = nc.dram_tensor("wpA", [P, NCT, NHP, P], bf16,
                         kind="ExternalInput")
    w1A = nc.dram_tensor("w1A", [P, NHT // 4, NCT, 512], bf16,
                         kind="ExternalInput")
    w2A = nc.dram_tensor("w2A", [P, NHT // 4, 2, 4, 512], bf16,
                         kind="ExternalInput")
    # all per-feature params packed host-side into one [P, 80] array:
    # cols 0:8 ln1_g, 8:16 ln1_b, 16:24 ln2_g, 24:32 ln2_b, 32:40 b_proj,
    # 40:72 b_fc1, 72:80 b_fc2
    pparams = nc.dram_tensor("pparams", [P, 80], f32, kind="ExternalInput")
    outT = nc.dram_tensor("outT", [C, TQ], f32, kind="ExternalOutput")

    with tile.TileContext(nc) as tc:
      with (
          tc.tile_pool(name="const", bufs=1) as const,
          tc.tile_pool(name="outer", bufs=1) as outer,
      ):
        onesf_row = const.tile([P, 256], f32)
        nc.vector.memset(onesf_row[:], 1.0)
        ones_row = const.tile([P, 256], bf16)
        nc.vector.tensor_copy(ones_row[:], onesf_row[:])
        ones_bf = const.tile([P, 1], bf16)
        nc.vector.tensor_copy(ones_bf[:], onesf_row[:, 0:1])
        ones32 = const.tile([P, 1], f32r)
        nc.vector.tensor_copy(ones32[:], onesf_row[:, 0:1])
        ones32_mm = const.tile([P, P], f32r)
        nc.vector.tensor_copy(ones32_mm[:], onesf_row[:, 0:P])
        eps_t = const.tile([P, 1], f32)
        nc.vector.memset(eps_t[:], EPS)

        pp_t = const.tile([P, 80], f32)
        nc.sync.dma_start(pp_t[:], pparams[:])

        x2T = outer.tile([P, NCT, TQ], f32r)  # attn residual output
        mrs2 = outer.tile([P, 2 * TQ], f32)   # LN2 mu | rstd, broadcast

        def ln_stats(ps_pool, sb_pool, src_ap, F):
            """Stage 1: load x chunk, emit squares + stats matmuls."""
            xt = sb_pool.tile([P, NCT, F], bf16, tag="xkv", bufs=3,
                              name="x_t")
            for ct in range(NCT):
                nc.sync.dma_start(xt[:, ct, :], src_ap[:, ct])
            ps_stat = ps_pool.tile([P, 2 * TQ], f32, tag="big", bufs=2,
                                   name="ps_stat")
            for ct in range(NCT):
                sq = sb_pool.tile([P, F], bf16, tag="ln_sq", bufs=3)
                nc.scalar.activation(sq[:], xt[:, ct, :], Act.Square)
                nc.tensor.matmul(ps_stat[0:1, 0:F], ones_bf[:], xt[:, ct, :],
                                 start=(ct == 0), stop=(ct == NCT - 1))
                nc.tensor.matmul(ps_stat[0:1, TQ:TQ + F], ones_bf[:], sq[:],
                                 start=(ct == 0), stop=(ct == NCT - 1))
            return xt, ps_stat

        def ln_smalls(sb_pool, ps_stat, F):
            """Stage 2: mu/var tiny ops + rstd via ACT ln/exp. Emitted
            ahead of the previous chunk's normalize so these 1-lane ops
            don't queue behind 24 big DVE ops."""
            mu = sb_pool.tile([1, F], f32r, tag="ln_mu", bufs=3)
            var = sb_pool.tile([1, F], f32r, tag="ln_var", bufs=3)
            nc.vector.tensor_scalar_mul(mu[:], ps_stat[0:1, 0:F], 1.0 / C)
            nc.vector.tensor_scalar_mul(var[:], ps_stat[0:1, TQ:TQ + F],
                                        1.0 / C)
            mu2 = sb_pool.tile([1, F], f32r, tag="ln_mu2", bufs=3)
            nc.vector.tensor_mul(mu2[:], mu[:], mu[:])
            nc.vector.tensor_sub(var[:], var[:], mu2[:])
            # rstd = exp(-0.5 * ln(var + eps))
            nc.scalar.activation(var[:], var[:], Act.Ln, bias=eps_t[0:1, :])
            nc.scalar.activation(var[:], var[:], Act.Exp, scale=-0.5)
            return mu, var

        def ln_apply(ps_pool, sb_pool, xt, mu, var, F, dst_name):
            """Stage 3: PE ones-product broadcast of mu/rstd, normalize."""
            ps_bc = ps_pool.tile([P, 2 * TQ], f32, tag="bc", bufs=1,
                                 name="ps_bc")
            nc.tensor.matmul(ps_bc[:, 0:F], ones32_mm[0:1, :], mu[:],
                             start=True, stop=True)
            nc.tensor.matmul(ps_bc[:, TQ:TQ + F], ones32_mm[0:1, :], var[:],
                             start=True, stop=True)
            mrs = sb_pool.tile([P, 2 * TQ], bf16, tag="ln_mrs", bufs=2,
                               name="mrs")
            nc.vector.tensor_copy(mrs[:], ps_bc[:])
            ln = sb_pool.tile([P, NCT, F], bf16, tag="lnkv", bufs=2,
                              name=dst_name)
            for ct in range(NCT):
                o = ln[:, ct, :]
                tmp = sb_pool.tile([P, F], bf16, tag="ln_tmp", bufs=3)
                nc.vector.tensor_sub(tmp[:], xt[:, ct, :], mrs[:, 0:F])
                nc.vector.tensor_mul(tmp[:], tmp[:], mrs[:, TQ:TQ + F])
                nc.vector.tensor_scalar(o, tmp[:], pp_t[:, 0 + ct:1 + ct],
                                        pp_t[:, 8 + ct:9 + ct],
                                        op0=mybir.AluOpType.mult,
                                        op1=mybir.AluOpType.add)
            return ln

        with tc.tile_pool(name="pa_big", bufs=1) as pa_big:
            kT = pa_big.tile([P, NHP, TKV], bf16)      # K, pair-dim major
            v_c = pa_big.tile([P, NJL, H, HD + 1], bf16)
            qT = pa_big.tile([P, NHP, TQ], bf16)
            attnT = pa_big.tile([P, NHP, TQ], bf16)    # normalized AV
            den = pa_big.tile([P, TQ], f32r)  # head A -> p0, head B -> p64
            rden = pa_big.tile([P, TQ], f32r)
            nc.vector.tensor_copy(den[:, 0:256], onesf_row[:])
            nc.vector.tensor_copy(den[:, 256:TQ], onesf_row[:])

            # ones row for denominators
            nc.vector.tensor_copy(
                v_c[:, :, :, HD],
                ones_row[:].rearrange("p (a b) -> p a b", a=NJL))

            # ---- Phase B1: LN1 + Q, then per chunk LN1 -> K -> V ----
            with (
                tc.tile_pool(name="pb1", bufs=1) as pb1,
                tc.tile_pool(name="ps1", bufs=1, space="PSUM") as pa_ps,
            ):
                wv_t = pb1.tile([P, NCT, C], bf16, name="wv_t")
                # issue on the ACT DMA queue so it doesn't serialize behind
                # the startup x-chunk loads on the sync queue
                for ct in range(NCT):
                    nc.scalar.dma_start(wv_t[:, ct, :], wvA[:, ct])

                def emit_k(lnkv, j0):
                    for hp in range(NHP):
                        wk_t = pb1.tile([P, NCT, P], bf16, tag="wk", bufs=2,
                                        name="wk_t")
                        nc.sync.dma_start(wk_t[:], wkA[:, hp])
                        ps = pa_ps.tile([P, CHUNK], f32, tag="kvq", bufs=2,
                                        name="k_ps")
                        for ct in range(NCT):
                            nc.tensor.matmul(ps[:], wk_t[:, ct, :],
                                             lnkv[:, ct, :],
                                             start=(ct == 0),
                                             stop=(ct == NCT - 1))
                        nc.scalar.copy(kT[:, hp, j0:j0 + CHUNK], ps[:])

                def emit_v(lnkv, ch):
                    for half in range(2):
                        for jloc in range(4):
                            jl = ch * 4 + jloc
                            ps = pa_ps.tile([P, CHUNK], f32, tag="kvq",
                                            bufs=2, name="v_ps")
                            for ct in range(NCT):
                                nc.tensor.matmul(
                                    ps[:],
                                    lnkv[:, ct, jloc * P:(jloc + 1) * P],
                                    wv_t[:, ct,
                                         half * CHUNK:(half + 1) * CHUNK],
                                    start=(ct == 0), stop=(ct == NCT - 1))
                            nc.vector.tensor_copy(
                                v_c[:, jl, half * 8:(half + 1) * 8, 0:HD],
                                ps[:].rearrange("p (h d) -> p h d", d=HD))

                def ln_chunk(src_ap, F, dst_name):
                    xt, ps_stat = ln_stats(pa_ps, pb1, src_ap, F)
                    mu, var = ln_smalls(pb1, ps_stat, F)
                    return ln_apply(pa_ps, pb1, xt, mu, var, F, dst_name)

                def emit_q():
                    # LN1 of the query block -> Q projection (emitted after
                    # chunk 1 so its serial LN chain hides under K/V work)
                    ln1q = ln_chunk(xqbA[:], TQ, "ln1q")
                    for hp in range(NHP):
                        wq_t = pb1.tile([P, NCT, P], bf16, tag="wk", bufs=2,
                                        name="wq_t")
                        nc.sync.dma_start(wq_t[:], wqA[:, hp])
                        ps = pa_ps.tile([P, TQ], f32, tag="kvq", bufs=2,
                                        name="q_ps")
                        for ct in range(NCT):
                            nc.tensor.matmul(ps[:], wq_t[:, ct, :],
                                             ln1q[:, ct, :],
                                             start=(ct == 0),
                                             stop=(ct == NCT - 1))
                        nc.scalar.copy(qT[:, hp, :], ps[:])

                # 3-stage pipelined LN1: the next chunk's stats (PE) and
                # smalls (DVE/ACT) are emitted before the current chunk's
                # normalize, so the broadcast matmul never waits
                xt0, ps0 = ln_stats(pa_ps, pb1, xkvA[:, 0], CHUNK)
                hold = (xt0, ln_smalls(pb1, ps0, CHUNK))
                for ch in range(NCHUNK):
                    if ch + 1 < NCHUNK:
                        xtn, psn = ln_stats(pa_ps, pb1, xkvA[:, ch + 1],
                                            CHUNK)
                        nxt = (xtn, ln_smalls(pb1, psn, CHUNK))
                    xt, (mu, var) = hold
                    lnkv = ln_apply(pa_ps, pb1, xt, mu, var, CHUNK, "lnkv")
                    emit_k(lnkv, ch * CHUNK)
                    emit_v(lnkv, ch)
                    if ch == 1:
                        emit_q()
                    if ch + 1 < NCHUNK:
                        hold = nxt

            # ---- Phase B2: attention (hp-outer), then proj + LN2 ----
            with (
                tc.tile_pool(name="pb2", bufs=1) as pb2,
                tc.tile_pool(name="ps2", bufs=1, space="PSUM") as pa_ps,
            ):
                xq = pb2.tile([P, NCT, TQ], f32r, name="xq")
                for ct in range(NCT):
                    nc.sync.dma_start(xq[:, ct, :], xqA[:, ct])
                wp_t = pb2.tile([P, NCT, NHP, P], bf16, name="wp_t")
                for ct in range(NCT):
                    nc.sync.dma_start(wp_t[:, ct], wpA[:, ct])

                def normalize(hp, ps_av0, ps_av1):
                    """Broadcast 1/den along partitions via a PE
                    ones-outer-product, then scale AV on DVE."""
                    ps_rcp = pa_ps.tile([P, 2 * TQ], f32, tag="big",
                                        bufs=2, name="ps_rcp")
                    nc.tensor.matmul(ps_rcp[:, 0:TQ], ones32_mm[0:1, :],
                                     rden[0:1, :], start=True, stop=True)
                    nc.tensor.matmul(ps_rcp[:, TQ:2 * TQ],
                                     ones32_mm[HD:HD + 1, :],
                                     rden[HD:HD + 1, :],
                                     start=True, stop=True)
                    rcp_sb = pb2.tile([P, 2 * TQ], f32, tag="rcpb", bufs=2,
                                      name="rcp_sb")
                    nc.vector.tensor_copy(rcp_sb[:], ps_rcp[:])
                    for i, ps_av in ((0, ps_av0), (1, ps_av1)):
                        nc.vector.tensor_mul(
                            attnT[i * HD:(i + 1) * HD, hp, :],
                            ps_av[0:HD, :],
                            rcp_sb[0:HD, i * TQ:(i + 1) * TQ])

                pend = None
                for hp in range(NHP):
                    ps_av0 = pa_ps.tile([HD + 1, TQ], f32, tag="av0",
                                        bufs=2, name="ps_av0")
                    ps_av1 = pa_ps.tile([HD + 1, TQ], f32, tag="av1",
                                        bufs=2, name="ps_av1")
                    for jl in range(NJL):
                        ps_sc = pa_ps.tile([P, 2 * TQ], f32, tag="big",
                                           bufs=2, name="ps_sc")
                        nc.tensor.matmul(
                            ps_sc[:, 0:TQ],
                            kT[0:HD, hp, jl * P:(jl + 1) * P],
                            qT[0:HD, hp, :], start=True, stop=True)
                        nc.tensor.matmul(
                            ps_sc[:, TQ:2 * TQ],
                            kT[HD:P, hp, jl * P:(jl + 1) * P],
                            qT[HD:P, hp, :], start=True, stop=True)
                        e_sb = pb2.tile([P, 2 * TQ], bf16, tag="e", bufs=3,
                                        name="e_sb")
                        nc.scalar.activation(e_sb[:], ps_sc[:], Act.Exp,
                                             scale=SCALE)
                        nc.tensor.matmul(
                            ps_av0[:], v_c[:, jl, 2 * hp, :], e_sb[:, 0:TQ],
                            start=(jl == 0), stop=(jl == NJL - 1))
                        nc.tensor.matmul(
                            ps_av1[:], v_c[:, jl, 2 * hp + 1, :],
                            e_sb[:, TQ:2 * TQ],
                            start=(jl == 0), stop=(jl == NJL - 1))
                        if jl == 4 and pend is not None:
                            normalize(*pend)
                            pend = None
                    # denominators: head A -> partition 0, head B -> 64
                    nc.vector.tensor_copy(den[0:1, :], ps_av0[HD:HD + 1, :])
                    nc.vector.tensor_copy(den[HD:HD + 1, :],
                                          ps_av1[HD:HD + 1, :])
                    with nc.allow_low_precision(
                            reason="f32r is bit-identical to f32"):
                        nc.vector.reciprocal(rden[:], den[:])
                    pend = (hp, ps_av0, ps_av1)
                normalize(*pend)

                # proj + residual + LN2 stats (stat accumulators reuse the
                # av banks, which are free once hp7 is normalized)
                ps_st2a = pa_ps.tile([HD + 1, TQ], f32, tag="av0", bufs=2,
                                     name="ps_st2a")
                ps_st2b = pa_ps.tile([HD + 1, TQ], f32, tag="av1", bufs=2,
                                     name="ps_st2b")
                for ct in range(NCT):
                    ps_b = pa_ps.tile([P, 2 * TQ], f32, tag="big", bufs=2,
                                      name="proj_ps")
                    ps = ps_b[:, 0:TQ]
                    for hp in range(NHP):
                        nc.tensor.matmul(ps[:], wp_t[:, ct, hp, :],
                                         attnT[:, hp, :],
                                         start=(hp == 0),
                                         stop=(hp == NHP - 1))
                    o = x2T[:, ct, :]
                    nc.scalar.activation(o, ps[:], Act.Identity,
                                         bias=pp_t[:, 32 + ct:33 + ct])
                    nc.vector.tensor_add(o, o, xq[:, ct, :])
                    sq2 = pb2.tile([P, TQ], bf16, tag="sq2", bufs=3,
                                   name="sq2")
                    nc.scalar.activation(sq2[:], o, Act.Square)
                    nc.tensor.matmul(ps_st2a[0:1, :], ones32[:], o,
                                     start=(ct == 0), stop=(ct == NCT - 1))
                    nc.tensor.matmul(ps_st2b[0:1, :], ones_bf[:], sq2[:],
                                     start=(ct == 0), stop=(ct == NCT - 1))

                mu = pb2.tile([1, TQ], f32r, tag="ln_mu", bufs=1, name="mu2")
                var = pb2.tile([1, TQ], f32r, tag="ln_var", bufs=1,
                               name="var2")
                nc.vector.tensor_scalar_mul(mu[:], ps_st2a[0:1, :], 1.0 / C)
                nc.vector.tensor_scalar_mul(var[:], ps_st2b[0:1, :], 1.0 / C)
                mu2_t = pb2.tile([1, TQ], f32r, tag="ln_mu2", bufs=1,
                                 name="mu2sq")
                nc.vector.tensor_mul(mu2_t[:], mu[:], mu[:])
                nc.vector.tensor_sub(var[:], var[:], mu2_t[:])
                nc.scalar.activation(var[:], var[:], Act.Ln,
                                     bias=eps_t[0:1, :])
                nc.scalar.activation(var[:], var[:], Act.Exp, scale=-0.5)
                ps_bc2 = pa_ps.tile([P, 2 * TQ], f32, tag="big", bufs=2,
                                    name="ps_bc2")
                nc.tensor.matmul(ps_bc2[:, 0:TQ], ones32_mm[0:1, :], mu[:],
                                 start=True, stop=True)
                nc.tensor.matmul(ps_bc2[:, TQ:2 * TQ], ones32_mm[0:1, :],
                                 var[:], start=True, stop=True)
                nc.vector.tensor_copy(mrs2[:], ps_bc2[:])

        # ---- Phase D: LN2 normalize, fc1+gelu, fc2 + residual ----
        with (
            tc.tile_pool(name="pd_sb", bufs=1) as pd_sb,
            tc.tile_pool(name="pd_g", bufs=1) as pd_g,
            tc.tile_pool(name="pd_w", bufs=1) as pd_w,
            tc.tile_pool(name="pd_ps", bufs=1, space="PSUM") as pd_ps,
            tc.tile_pool(name="pd_ps2", bufs=1, space="PSUM") as pd_ps2,
        ):
            ln2T = pd_g.tile([P, NCT, TQ], bf16, name="ln2T")
            for ct in range(NCT):
                o = ln2T[:, ct, :]
                tmp = pd_sb.tile([P, TQ], f32, tag=f"l2t{ct % 2}", bufs=2,
                                 name="l2tmp")
                nc.vector.tensor_sub(tmp[:], x2T[:, ct, :], mrs2[:, 0:TQ])
                nc.vector.tensor_mul(tmp[:], tmp[:], mrs2[:, TQ:2 * TQ])
                nc.vector.tensor_scalar(o, tmp[:], pp_t[:, 16 + ct:17 + ct],
                                        pp_t[:, 24 + ct:25 + ct],
                                        op0=mybir.AluOpType.mult,
                                        op1=mybir.AluOpType.add)

            g1T = pd_g.tile([P, NHT, TQ], bf16, name="g1T")
            for htg in range(NHT // 4):
                w1_t = pd_w.tile([P, NCT, 512], bf16, tag="w1", bufs=2,
                                 name="w1_t")
                for ct2 in range(0, NCT, 2):
                    nc.sync.dma_start(w1_t[:, ct2:ct2 + 2, :],
                                      w1A[:, htg, ct2:ct2 + 2])
                for hl in range(4):
                    ht = htg * 4 + hl
                    ps = pd_ps.tile([P, TQ], f32, tag="fc1_ps", bufs=2,
                                    name="fc1_ps")
                    for ct in range(NCT):
                        nc.tensor.matmul(
                            ps[:], w1_t[:, ct, hl * P:(hl + 1) * P],
                            ln2T[:, ct, :],
                            start=(ct == 0), stop=(ct == NCT - 1))
                    nc.scalar.activation(g1T[:, ht, :], ps[:], gelu_fn,
                                         bias=pp_t[:, 40 + ht:41 + ht])

            for ctg in range(2):
                ps_out = [pd_ps2.tile([P, TQ], f32, tag=f"fc2_{i}",
                                      name=f"fc2_ps_{i}")
                          for i in range(4)]
                for htg4 in range(NHT // 4):
                    w2_t = pd_w.tile([P, 4, 512], bf16, tag="w2", bufs=3,
                                     name="w2_t")
                    for h2 in range(0, 4, 2):
                        nc.sync.dma_start(w2_t[:, h2:h2 + 2, :],
                                          w2A[:, htg4, ctg, h2:h2 + 2])
                    for hl in range(4):
                        ht = htg4 * 4 + hl
                        for cl in range(4):
                            nc.tensor.matmul(
                                ps_out[cl][:],
                                w2_t[:, hl, cl * P:(cl + 1) * P],
                                g1T[:, ht, :],
                                start=(ht == 0), stop=(ht == NHT - 1))
                for cl in range(4):
                    ct = ctg * 4 + cl
                    o = pd_sb.tile([P, TQ], f32, tag="out_t", bufs=3,
                                   name="out_t")
                    nc.scalar.activation(o[:], ps_out[cl][:], Act.Identity,
                                         bias=pp_t[:, 72 + ct:73 + ct])
                    nc.vector.tensor_add(o[:], o[:], x2T[:, ct, :])
                    nc.sync.dma_start(outT[ct * P:(ct + 1) * P, :], o[:])

    nc.finalize()
    return nc


_program = None

# test.py can set RUN_OPTS (e.g. trace=True) and read LAST_RESULTS for
# exec_time_ns / trace paths. The grading harness uses neither.
RUN_OPTS = {}
LAST_RESULTS = None


def _get_program():
    global _program
    if _program is None:
        _program = build_program()
    return _program


def pack_params(inputs):
    """Pack per-feature params into [P, 80] (see pparams layout comment)."""
    def cols(name, n):
        return np.asarray(inputs[name], np.float32).reshape(n // P, P).T
    return np.ascontiguousarray(np.concatenate([
        cols("ln1_g", C), cols("ln1_b", C), cols("ln2_g", C),
        cols("ln2_b", C), cols("b_proj", C), cols("b_fc1", HID),
        cols("b_fc2", C)], axis=1))


def make_in_maps(inputs):
    import ml_dtypes

    bf = ml_dtypes.bfloat16
    x = np.asarray(inputs["x"], dtype=np.float32)
    B, N, _ = x.shape  # [2, 2048, 1024]

    w_qkv = np.asarray(inputs["w_qkv"], dtype=np.float32)

    def arr(a):
        return np.ascontiguousarray(a)

    # weights pre-arranged into on-chip tile layouts (see dram_tensor
    # comments); all are [P, ...] with contiguous per-partition rows
    wqT = w_qkv[0:C].T
    wkT = w_qkv[C:2 * C].T
    wvT = w_qkv[2 * C:3 * C].T
    wpT = np.asarray(inputs["w_proj"], np.float32).T
    w1T = np.asarray(inputs["w_fc1"], np.float32).T
    w2T = np.asarray(inputs["w_fc2"], np.float32).T
    shared = {
        "wqA": arr(wqT.reshape(8, P, 8, P).transpose(1, 2, 0, 3).astype(bf)),
        "wkA": arr(wkT.reshape(8, P, 8, P).transpose(1, 2, 0, 3).astype(bf)),
        "wvA": arr(wvT.reshape(8, P, C).transpose(1, 0, 2).astype(bf)),
        "wpA": arr(wpT.reshape(8, P, 8, P).transpose(1, 2, 0, 3).astype(bf)),
        "w1A": arr(w1T.reshape(8, P, 8, 512).transpose(1, 2, 0, 3)
                   .astype(bf)),
        "w2A": arr(w2T.reshape(8, 4, P, 2, 512).transpose(2, 0, 3, 1, 4)
                   .astype(bf)),
        "pparams": pack_params(inputs),
    }
    in_maps = []
    for core in range(8):
        b, qb = core // 4, core % 4
        xT = x[b].T  # [C, N] f32
        xq = xT[:, qb * TQ:(qb + 1) * TQ]
        m = dict(shared)
        m["xkvA"] = arr(xT.reshape(8, P, 4, 512).transpose(1, 2, 0, 3)
                        .astype(bf))
        m["xqbA"] = arr(xq.reshape(8, P, TQ).transpose(1, 0, 2).astype(bf))
        m["xqA"] = arr(xq.reshape(8, P, TQ).transpose(1, 0, 2)
                       .astype(np.float32))
        in_maps.append(m)
    return in_maps


def kernel(**inputs):
    in_maps = make_in_maps(inputs)
    x = np.asarray(inputs["x"], dtype=np.float32)
    B, N, _ = x.shape

    nc = _get_program()
    global LAST_RESULTS
    res = run_bass_kernel_spmd(nc, in_maps, list(range(8)), **RUN_OPTS)
    LAST_RESULTS = res

    out = np.empty((B, N, C), dtype=np.float32)
    for core in range(8):
        b, qb = core // 4, core % 4
        out[b, qb * TQ:(qb + 1) * TQ, :] = res.results[core]["outT"].T
    return out


# revision 54
# speedup vs baseline: 1.1505x; 1.1505x over previous
"""Trainium2 Bass kernel for a dense transformer block (pre-LN, MHA + MLP).

Sharding: 8 cores; core c handles batch b = c // 4, query block qb = c % 4
(512 tokens). Each core recomputes K/V for its batch's full 2048-token
sequence (zero cross-core communication), then runs attention for its
512 query tokens and the MLP on them.

All matmul inputs are bf16 (residual path stays f32); every weight is
loaded once. The host pre-arranges all inputs into the exact on-chip tile
layout ([P, ...] per-partition contiguous rows), split into per-ct
sub-DMAs for DMA-engine parallelism. Phase B1 streams 512-token chunks:
LN1 -> K proj (all 8 head pairs) -> V proj, so lnkv is a rotating 2-buffer
stream. Phase B2 is head-pair-outer attention with AV accumulated across
all 16 kv subchunks in PSUM (ACT-exp-bound at ~96% occupancy, the floor
for this sharding); the two K=64 score matmuls of a head pair go to PE
row groups 0/64 and run concurrently (PE row tiling). LayerNorm rstd uses
exp(-0.5*ln(var+eps)) on ACT; mu/rstd and softmax 1/den broadcasts along
partitions use PE ones-outer-products into PSUM instead of slow gpsimd
partition_broadcast; softmax denominators get one batched DVE reciprocal
per head pair, deferred into the next head pair's score loop so the PE
never waits on it.
"""
import numpy as np

import concourse.bass as bass
import concourse.mybir as mybir
import concourse.tile as tile
from concourse import bacc
from concourse.bass_utils import run_bass_kernel_spmd

P = 128
C = 1024
NCT = C // P          # 8 feature tiles
TKV = 2048            # kv tokens per core (sequence length)
TQ = 512              # query tokens per core
HID = 4096
NHT = HID // P        # 32 hidden tiles
H = 16
HD = 64
NHP = H // 2          # 8 head pairs
NJL = TKV // P        # 16 kv subchunks of 128
CHUNK = 512           # ln1/K/V processing chunk
NCHUNK = TKV // CHUNK # 4
EPS = 1e-5
SCALE = HD ** -0.5

f32 = mybir.dt.float32
f32r = mybir.dt.float32r
bf16 = mybir.dt.bfloat16
Act = mybir.ActivationFunctionType


def build_program(sim_standin=False):
    # CoreSim lacks Gelu; Tanh has identical ACT cost, so the sim variant
    # swaps it in for modeled-time runs.
    gelu_fn = Act.Tanh if sim_standin else Act.Gelu
    nc = bacc.Bacc()

    # DRAM I/O (per core). Host pre-arranges everything into the exact
    # on-chip tile layout, so every DMA is per-partition contiguous
    # (128 descriptors instead of 1024+).
    xkvA = nc.dram_tensor("xkvA", [P, NCHUNK, NCT, CHUNK], bf16,
                          kind="ExternalInput")
    xqbA = nc.dram_tensor("xqbA", [P, NCT, TQ], bf16, kind="ExternalInput")
    xqA = nc.dram_tensor("xqA", [P, NCT, TQ], f32r, kind="ExternalInput")
    wqA = nc.dram_tensor("wqA", [P, NHP, NCT, P], bf16,
                         kind="ExternalInput")
    wkA = nc.dram_tensor("wkA", [P, NHP, NCT, P], bf16,
                         kind="ExternalInput")
    wvA = nc.dram_tensor("wvA", [P, NCT, C], bf16, kind="ExternalInput")
    wpA = nc.dram_tensor("wpA", [P, NCT, NHP, P], bf16,
                         kind="ExternalInput")
    w1A = nc.dram_tensor("w1A", [P, NHT // 4, NCT, 512], bf16,
                         kind="ExternalInput")
    w2A = nc.dram_tensor("w2A", [P, NHT // 4, 2, 4, 512], bf16,
                         kind="ExternalInput")
    # all per-feature params packed host-side into one [P, 80] array:
    # cols 0:8 ln1_g, 8:16 ln1_b, 16:24 ln2_g, 24:32 ln2_b, 32:40 b_proj,
    # 40:72 b_fc1, 72:80 b_fc2
    pparams = nc.dram_tensor("pparams", [P, 80], f32, kind="ExternalInput")
    outT = nc.dram_tensor("outT", [C, TQ], f32, kind="ExternalOutput")

    with tile.TileContext(nc) as tc:
      with (
          tc.tile_pool(name="const", bufs=1) as const,
          tc.tile_pool(name="outer", bufs=1) as outer,
      ):
        onesf_row = const.tile([P, 256], f32)
        nc.vector.memset(onesf_row[:], 1.0)
        ones_row = const.tile([P, 256], bf16)
        nc.vector.tensor_copy(ones_row[:], onesf_row[:])
        ones_bf = const.tile([P, 1], bf16)
        nc.vector.tensor_copy(ones_bf[:], onesf_row[:, 0:1])
        ones32 = const.tile([P, 1], f32r)
        nc.vector.tensor_copy(ones32[:], onesf_row[:, 0:1])
        ones32_mm = const.tile([P, P], f32r)
        nc.vector.tensor_copy(ones32_mm[:], onesf_row[:, 0:P])
        eps_t = const.tile([P, 1], f32)
        nc.vector.memset(eps_t[:], EPS)

        pp_t = const.tile([P, 80], f32)
        nc.sync.dma_start(pp_t[:], pparams[:])

        x2T = outer.tile([P, NCT, TQ], f32r)  # attn residual output
        mrs2 = outer.tile([P, 2 * TQ], f32)   # LN2 mu | rstd, broadcast

        def ln_stats(ps_pool, sb_pool, src_ap, F):
            """Stage 1: load x chunk, emit squares + stats matmuls."""
            xt = sb_pool.tile([P, NCT, F], bf16, tag="xkv", bufs=3,
                              name="x_t")
            for ct in range(NCT):
                nc.sync.dma_start(xt[:, ct, :], src_ap[:, ct])
            ps_stat = ps_pool.tile([P, 2 * TQ], f32, tag="big", bufs=2,
                                   name="ps_stat")
            for ct in range(NCT):
                sq = sb_pool.tile([P, F], bf16, tag="ln_sq", bufs=3)
                nc.scalar.activation(sq[:], xt[:, ct, :], Act.Square)
                nc.tensor.matmul(ps_stat[0:1, 0:F], ones_bf[:], xt[:, ct, :],
                                 start=(ct == 0), stop=(ct == NCT - 1))
                nc.tensor.matmul(ps_stat[0:1, TQ:TQ + F], ones_bf[:], sq[:],
                                 start=(ct == 0), stop=(ct == NCT - 1))
            return xt, ps_stat

        def ln_smalls(sb_pool, ps_stat, F):
            """Stage 2: mu/var tiny ops + rstd via ACT ln/exp. Emitted
            ahead of the previous chunk's normalize so these 1-lane ops
            don't queue behind 24 big DVE ops."""
            mu = sb_pool.tile([1, F], f32r, tag="ln_mu", bufs=3)
            var = sb_pool.tile([1, F], f32r, tag="ln_var", bufs=3)
            nc.vector.tensor_scalar_mul(mu[:], ps_stat[0:1, 0:F], 1.0 / C)
            nc.vector.tensor_scalar_mul(var[:], ps_stat[0:1, TQ:TQ + F],
                                        1.0 / C)
            mu2 = sb_pool.tile([1, F], f32r, tag="ln_mu2", bufs=3)
            nc.vector.tensor_mul(mu2[:], mu[:], mu[:])
            nc.vector.tensor_sub(var[:], var[:], mu2[:])
            # rstd = exp(-0.5 * ln(var + eps))
            nc.scalar.activation(var[:], var[:], Act.Ln, bias=eps_t[0:1, :])
            nc.scalar.activation(var[:], var[:], Act.Exp, scale=-0.5)
            return mu, var

        def ln_apply(ps_pool, sb_pool, xt, mu, var, F, dst_name):
            """Stage 3: PE ones-product broadcast of mu/rstd, normalize."""
            ps_bc = ps_pool.tile([P, 2 * TQ], f32, tag="bc", bufs=1,
                                 name="ps_bc")
            nc.tensor.matmul(ps_bc[:, 0:F], ones32_mm[0:1, :], mu[:],
                             start=True, stop=True)
            nc.tensor.matmul(ps_bc[:, TQ:TQ + F], ones32_mm[0:1, :], var[:],
                             start=True, stop=True)
            mrs = sb_pool.tile([P, 2 * TQ], bf16, tag="ln_mrs", bufs=2,
                               name="mrs")
            nc.vector.tensor_copy(mrs[:], ps_bc[:])
            ln = sb_pool.tile([P, NCT, F], bf16, tag="lnkv", bufs=2,
                              name=dst_name)
            for ct in range(NCT):
                o = ln[:, ct, :]
                tmp = sb_pool.tile([P, F], bf16, tag="ln_tmp", bufs=3)
                nc.vector.tensor_sub(tmp[:], xt[:, ct, :], mrs[:, 0:F])
                nc.vector.tensor_mul(tmp[:], tmp[:], mrs[:, TQ:TQ + F])
                nc.vector.tensor_scalar(o, tmp[:], pp_t[:, 0 + ct:1 + ct],
                                        pp_t[:, 8 + ct:9 + ct],
                                        op0=mybir.AluOpType.mult,
                                        op1=mybir.AluOpType.add)
            return ln

        with tc.tile_pool(name="pa_big", bufs=1) as pa_big:
            kT = pa_big.tile([P, NHP, TKV], bf16)      # K, pair-dim major
            v_c = pa_big.tile([P, NJL, H, HD + 1], bf16)
            qT = pa_big.tile([P, NHP, TQ], bf16)
            attnT = pa_big.tile([P, NHP, TQ], bf16)    # normalized AV
            den = pa_big.tile([P, TQ], f32r)  # head A -> p0, head B -> p64
            rden = pa_big.tile([P, TQ], f32r)
            nc.vector.tensor_copy(den[:, 0:256], onesf_row[:])
            nc.vector.tensor_copy(den[:, 256:TQ], onesf_row[:])

            # ones row for denominators
            nc.vector.tensor_copy(
                v_c[:, :, :, HD],
                ones_row[:].rearrange("p (a b) -> p a b", a=NJL))

            # ---- Phase B1: LN1 + Q, then per chunk LN1 -> K -> V ----
            with (
                tc.tile_pool(name="pb1", bufs=1) as pb1,
                tc.tile_pool(name="ps1", bufs=1, space="PSUM") as pa_ps,
            ):
                wv_t = pb1.tile([P, NCT, C], bf16, name="wv_t")
                # issue on the ACT DMA queue so it doesn't serialize behind
                # the startup x-chunk loads on the sync queue
                for ct in range(NCT):
                    nc.scalar.dma_start(wv_t[:, ct, :], wvA[:, ct])

                def emit_k(lnkv, j0):
                    for hp in range(NHP):
                        wk_t = pb1.tile([P, NCT, P], bf16, tag="wk", bufs=2,
                                        name="wk_t")
                        nc.sync.dma_start(wk_t[:], wkA[:, hp])
                        ps = pa_ps.tile([P, CHUNK], f32, tag="kvq", bufs=2,
                                        name="k_ps")
                        for ct in range(NCT):
                            nc.tensor.matmul(ps[:], wk_t[:, ct, :],
                                             lnkv[:, ct, :],
                                             start=(ct == 0),
                                             stop=(ct == NCT - 1))
                        nc.scalar.copy(kT[:, hp, j0:j0 + CHUNK], ps[:])

                def emit_v(lnkv, ch):
                    for half in range(2):
                        for jloc in range(4):
                            jl = ch * 4 + jloc
                            ps = pa_ps.tile([P, CHUNK], f32, tag="kvq",
                                            bufs=2, name="v_ps")
                            for ct in range(NCT):
                                nc.tensor.matmul(
                                    ps[:],
                                    lnkv[:, ct, jloc * P:(jloc + 1) * P],
                                    wv_t[:, ct,
                                         half * CHUNK:(half + 1) * CHUNK],
                                    start=(ct == 0), stop=(ct == NCT - 1))
                            nc.vector.tensor_copy(
                                v_c[:, jl, half * 8:(half + 1) * 8, 0:HD],
                                ps[:].rearrange("p (h d) -> p h d", d=HD))

                def ln_chunk(src_ap, F, dst_name):
                    xt, ps_stat = ln_stats(pa_ps, pb1, src_ap, F)
                    mu, var = ln_smalls(pb1, ps_stat, F)
                    return ln_apply(pa_ps, pb1, xt, mu, var, F, dst_name)

                def emit_q():
                    # LN1 of the query block -> Q projection (emitted after
                    # chunk 1 so its serial LN chain hides under K/V work)
                    ln1q = ln_chunk(xqbA[:], TQ, "ln1q")
                    for hp in range(NHP):
                        wq_t = pb1.tile([P, NCT, P], bf16, tag="wk", bufs=2,
                                        name="wq_t")
                        nc.sync.dma_start(wq_t[:], wqA[:, hp])
                        ps = pa_ps.tile([P, TQ], f32, tag="kvq", bufs=2,
                                        name="q_ps")
                        for ct in range(NCT):
                            nc.tensor.matmul(ps[:], wq_t[:, ct, :],
                                             ln1q[:, ct, :],
                                             start=(ct == 0),
                                             stop=(ct == NCT - 1))
                        nc.scalar.copy(qT[:, hp, :], ps[:])

                for ch in range(NCHUNK):
                    lnkv = ln_chunk(xkvA[:, ch], CHUNK, "lnkv")
                    emit_k(lnkv, ch * CHUNK)
                    emit_v(lnkv, ch)
                    if ch == 1:
                        emit_q()

            # ---- Phase B2: attention (hp-outer), then proj + LN2 ----
            with (
                tc.tile_pool(name="pb2", bufs=1) as pb2,
                tc.tile_pool(name="ps2", bufs=1, space="PSUM") as pa_ps,
            ):
                xq = pb2.tile([P, NCT, TQ], f32r, name="xq")
                for ct in range(NCT):
                    nc.sync.dma_start(xq[:, ct, :], xqA[:, ct])
                wp_t = pb2.tile([P, NCT, NHP, P], bf16, name="wp_t")
                for ct in range(NCT):
                    nc.sync.dma_start(wp_t[:, ct], wpA[:, ct])

                def normalize(hp, ps_av0, ps_av1):
                    """Broadcast 1/den along partitions via a PE
                    ones-outer-product, then scale AV on DVE."""
                    ps_rcp = pa_ps.tile([P, 2 * TQ], f32, tag="big",
                                        bufs=2, name="ps_rcp")
                    nc.tensor.matmul(ps_rcp[:, 0:TQ], ones32_mm[0:1, :],
                                     rden[0:1, :], start=True, stop=True)
                    nc.tensor.matmul(ps_rcp[:, TQ:2 * TQ],
                                     ones32_mm[HD:HD + 1, :],
                                     rden[HD:HD + 1, :],
                                     start=True, stop=True)
                    rcp_sb = pb2.tile([P, 2 * TQ], f32, tag="rcpb", bufs=2,
                                      name="rcp_sb")
                    nc.vector.tensor_copy(rcp_sb[:], ps_rcp[:])
                    for i, ps_av in ((0, ps_av0), (1, ps_av1)):
                        nc.vector.tensor_mul(
                            attnT[i * HD:(i + 1) * HD, hp, :],
                            ps_av[0:HD, :],
                            rcp_sb[0:HD, i * TQ:(i + 1) * TQ])

                pend = None
                for hp in range(NHP):
                    ps_av0 = pa_ps.tile([HD + 1, TQ], f32, tag="av0",
                                        bufs=2, name="ps_av0")
                    ps_av1 = pa_ps.tile([HD + 1, TQ], f32, tag="av1",
                                        bufs=2, name="ps_av1")
                    for jl in range(NJL):
                        ps_sc = pa_ps.tile([P, 2 * TQ], f32, tag="big",
                                           bufs=2, name="ps_sc")
                        nc.tensor.matmul(
                            ps_sc[:, 0:TQ],
                            kT[0:HD, hp, jl * P:(jl + 1) * P],
                            qT[0:HD, hp, :], start=True, stop=True)
                        nc.tensor.matmul(
                            ps_sc[:, TQ:2 * TQ],
                            kT[HD:P, hp, jl * P:(jl + 1) * P],
                            qT[HD:P, hp, :], start=True, stop=True)
                        e_sb = pb2.tile([P, 2 * TQ], bf16, tag="e", bufs=3,
                                        name="e_sb")
                        nc.scalar.activation(e_sb[:], ps_sc[:], Act.Exp,
                                             scale=SCALE)
                        nc.tensor.matmul(
                            ps_av0[:], v_c[:, jl, 2 * hp, :], e_sb[:, 0:TQ],
                            start=(jl == 0), stop=(jl == NJL - 1))
                        nc.tensor.matmul(
                            ps_av1[:], v_c[:, jl, 2 * hp + 1, :],
                            e_sb[:, TQ:2 * TQ],
                            start=(jl == 0), stop=(jl == NJL - 1))
                        if jl == 4 and pend is not None:
                            normalize(*pend)
                            pend = None
                    # denominators: head A -> partition 0, head B -> 64
                    nc.vector.tensor_copy(den[0:1, :], ps_av0[HD:HD + 1, :])
                    nc.vector.tensor_copy(den[HD:HD + 1, :],
                                          ps_av1[HD:HD + 1, :])
                    with nc.allow_low_precision(
                            reason="f32r is bit-identical to f32"):
                        nc.vector.reciprocal(rden[:], den[:])
                    pend = (hp, ps_av0, ps_av1)
                normalize(*pend)

                # proj + residual + LN2 stats (stat accumulators reuse the
                # av banks, which are free once hp7 is normalized)
                ps_st2a = pa_ps.tile([HD + 1, TQ], f32, tag="av0", bufs=2,
                                     name="ps_st2a")
                ps_st2b = pa_ps.tile([HD + 1, TQ], f32, tag="av1", bufs=2,
                                     name="ps_st2b")
                for ct in range(NCT):
                    ps_b = pa_ps.tile([P, 2 * TQ], f32, tag="big", bufs=2,
                                      name="proj_ps")
                    ps = ps_b[:, 0:TQ]
                    for hp in range(NHP):
                        nc.tensor.matmul(ps[:], wp_t[:, ct, hp, :],
                                         attnT[:, hp, :],
                                         start=(hp == 0),
                                         stop=(hp == NHP - 1))
                    o = x2T[:, ct, :]
                    nc.scalar.activation(o, ps[:], Act.Identity,
                                         bias=pp_t[:, 32 + ct:33 + ct])
                    nc.vector.tensor_add(o, o, xq[:, ct, :])
                    sq2 = pb2.tile([P, TQ], bf16, tag="sq2", bufs=3,
                                   name="sq2")
                    nc.scalar.activation(sq2[:], o, Act.Square)
                    nc.tensor.matmul(ps_st2a[0:1, :], ones32[:], o,
                                     start=(ct == 0), stop=(ct == NCT - 1))
                    nc.tensor.matmul(ps_st2b[0:1, :], ones_bf[:], sq2[:],
                                     start=(ct == 0), stop=(ct == NCT - 1))

                mu = pb2.tile([1, TQ], f32r, tag="ln_mu", bufs=1, name="mu2")
                var = pb2.tile([1, TQ], f32r, tag="ln_var", bufs=1,
                               name="var2")
                nc.vector.tensor_scalar_mul(mu[:], ps_st2a[0:1, :], 1.0 / C)
                nc.vector.tensor_scalar_mul(var[:], ps_st2b[0:1, :], 1.0 / C)
                mu2_t = pb2.tile([1, TQ], f32r, tag="ln_mu2", bufs=1,
                                 name="mu2sq")
                nc.vector.tensor_mul(mu2_t[:], mu[:], mu[:])
                nc.vector.tensor_sub(var[:], var[:], mu2_t[:])
                nc.scalar.activation(var[:], var[:], Act.Ln,
                                     bias=eps_t[0:1, :])
                nc.scalar.activation(var[:], var[:], Act.Exp, scale=-0.5)
                ps_bc2 = pa_ps.tile([P, 2 * TQ], f32, tag="big", bufs=2,
                                    name="ps_bc2")
                nc.tensor.matmul(ps_bc2[:, 0:TQ], ones32_mm[0:1, :], mu[:],
                                 start=True, stop=True)
                nc.tensor.matmul(ps_bc2[:, TQ:2 * TQ], ones32_mm[0:1, :],
                                 var[:], start=True, stop=True)
                nc.vector.tensor_copy(mrs2[:], ps_bc2[:])

        # ---- Phase D: LN2 normalize, fc1+gelu, fc2 + residual ----
        with (
            tc.tile_pool(name="pd_sb", bufs=1) as pd_sb,
            tc.tile_pool(name="pd_g", bufs=1) as pd_g,
            tc.tile_pool(name="pd_w", bufs=1) as pd_w,
            tc.tile_pool(name="pd_ps", bufs=1, space="PSUM") as pd_ps,
            tc.tile_pool(name="pd_ps2", bufs=1, space="PSUM") as pd_ps2,
        ):
            ln2T = pd_g.tile([P, NCT, TQ], bf16, name="ln2T")
            for ct in range(NCT):
                o = ln2T[:, ct, :]
                tmp = pd_sb.tile([P, TQ], f32, tag=f"l2t{ct % 2}", bufs=2,
                                 name="l2tmp")
                nc.vector.tensor_sub(tmp[:], x2T[:, ct, :], mrs2[:, 0:TQ])
                nc.vector.tensor_mul(tmp[:], tmp[:], mrs2[:, TQ:2 * TQ])
                nc.vector.tensor_scalar(o, tmp[:], pp_t[:, 16 + ct:17 + ct],
                                        pp_t[:, 24 + ct:25 + ct],
                                        op0=mybir.AluOpType.mult,
                                        op1=mybir.AluOpType.add)

            g1T = pd_g.tile([P, NHT, TQ], bf16, name="g1T")
            for htg in range(NHT // 4):
                w1_t = pd_w.tile([P, NCT, 512], bf16, tag="w1", bufs=2,
                                 name="w1_t")
                for ct2 in range(0, NCT, 2):
                    nc.sync.dma_start(w1_t[:, ct2:ct2 + 2, :],
                                      w1A[:, htg, ct2:ct2 + 2])
                for hl in range(4):
                    ht = htg * 4 + hl
                    ps = pd_ps.tile([P, TQ], f32, tag="fc1_ps", bufs=2,
                                    name="fc1_ps")
                    for ct in range(NCT):
                        nc.tensor.matmul(
                            ps[:], w1_t[:, ct, hl * P:(hl + 1) * P],
                            ln2T[:, ct, :],
                            start=(ct == 0), stop=(ct == NCT - 1))
                    nc.scalar.activation(g1T[:, ht, :], ps[:], gelu_fn,
                                         bias=pp_t[:, 40 + ht:41 + ht])

            for ctg in range(2):
                ps_out = [pd_ps2.tile([P, TQ], f32, tag=f"fc2_{i}",
                                      name=f"fc2_ps_{i}")
                          for i in range(4)]
                for htg4 in range(NHT // 4):
                    w2_t = pd_w.tile([P, 4, 512], bf16, tag="w2", bufs=3,
                                     name="w2_t")
                    for h2 in range(0, 4, 2):
                        nc.sync.dma_start(w2_t[:, h2:h2 + 2, :],
                                          w2A[:, htg4, ctg, h2:h2 + 2])
                    for hl in range(4):
                        ht = htg4 * 4 + hl
                        for cl in range(4):
                            nc.tensor.matmul(
                                ps_out[cl][:],
                                w2_t[:, hl, cl * P:(cl + 1) * P],
                                g1T[:, ht, :],
                                start=(ht == 0), stop=(ht == NHT - 1))
                for cl in range(4):
                    ct = ctg * 4 + cl
                    o = pd_sb.tile([P, TQ], f32, tag="out_t", bufs=3,
                                   name="out_t")
                    nc.scalar.activation(o[:], ps_out[cl][:], Act.Identity,
                                         bias=pp_t[:, 72 + ct:73 + ct])
                    nc.vector.tensor_add(o[:], o[:], x2T[:, ct, :])
                    nc.sync.dma_start(outT[ct * P:(ct + 1) * P, :], o[:])

    nc.finalize()
    return nc


_program = None

# test.py can set RUN_OPTS (e.g. trace=True) and read LAST_RESULTS for
# exec_time_ns / trace paths. The grading harness uses neither.
RUN_OPTS = {}
LAST_RESULTS = None


def _get_program():
    global _program
    if _program is None:
        _program = build_program()
    return _program


def pack_params(inputs):
    """Pack per-feature params into [P, 80] (see pparams layout comment)."""
    def cols(name, n):
        return np.asarray(inputs[name], np.float32).reshape(n // P, P).T
    return np.ascontiguousarray(np.concatenate([
        cols("ln1_g", C), cols("ln1_b", C), cols("ln2_g", C),
        cols("ln2_b", C), cols("b_proj", C), cols("b_fc1", HID),
        cols("b_fc2", C)], axis=1))


def make_in_maps(inputs):
    import ml_dtypes

    bf = ml_dtypes.bfloat16
    x = np.asarray(inputs["x"], dtype=np.float32)
    B, N, _ = x.shape  # [2, 2048, 1024]

    w_qkv = np.asarray(inputs["w_qkv"], dtype=np.float32)

    def arr(a):
        return np.ascontiguousarray(a)

    # weights pre-arranged into on-chip tile layouts (see dram_tensor
    # comments); all are [P, ...] with contiguous per-partition rows
    wqT = w_qkv[0:C].T
    wkT = w_qkv[C:2 * C].T
    wvT = w_qkv[2 * C:3 * C].T
    wpT = np.asarray(inputs["w_proj"], np.float32).T
    w1T = np.asarray(inputs["w_fc1"], np.float32).T
    w2T = np.asarray(inputs["w_fc2"], np.float32).T
    shared = {
        "wqA": arr(wqT.reshape(8, P, 8, P).transpose(1, 2, 0, 3).astype(bf)),
        "wkA": arr(wkT.reshape(8, P, 8, P).transpose(1, 2, 0, 3).astype(bf)),
        "wvA": arr(wvT.reshape(8, P, C).transpose(1, 0, 2).astype(bf)),
        "wpA": arr(wpT.reshape(8, P, 8, P).transpose(1, 2, 0, 3).astype(bf)),
        "w1A": arr(w1T.reshape(8, P, 8, 512).transpose(1, 2, 0, 3)
                   .astype(bf)),
        "w2A": arr(w2T.reshape(8, 4, P, 2, 512).transpose(2, 0, 3, 1, 4)
                   .astype(bf)),
        "pparams": pack_params(inputs),
    }
    in_maps = []
    for core in range(8):
        b, qb = core // 4, core % 4
        xT = x[b].T  # [C, N] f32
        xq = xT[:, qb * TQ:(qb + 1) * TQ]
        m = dict(shared)
        m["xkvA"] = arr(xT.reshape(8, P, 4, 512).transpose(1, 2, 0, 3)
                        .astype(bf))
        m["xqbA"] = arr(xq.reshape(8, P, TQ).transpose(1, 0, 2).astype(bf))
        m["xqA"] = arr(xq.reshape(8, P, TQ).transpose(1, 0, 2)
                       .astype(np.float32))
        in_maps.append(m)
    return in_maps


def kernel(**inputs):
    in_maps = make_in_maps(inputs)
    x = np.asarray(inputs["x"], dtype=np.float32)
    B, N, _ = x.shape

    nc = _get_program()
    global LAST_RESULTS
    res = run_bass_kernel_spmd(nc, in_maps, list(range(8)), **RUN_OPTS)
    LAST_RESULTS = res

    out = np.empty((B, N, C), dtype=np.float32)
    for core in range(8):
        b, qb = core // 4, core % 4
        out[b, qb * TQ:(qb + 1) * TQ, :] = res.results[core]["outT"].T
    return out


# revision 55
# speedup vs baseline: 1.1900x; 1.0344x over previous
"""Trainium2 Bass kernel for a dense transformer block (pre-LN, MHA + MLP).

Sharding: 8 cores; core c handles batch b = c // 4, query block qb = c % 4
(512 tokens). Each core recomputes K/V for its batch's full 2048-token
sequence (zero cross-core communication), then runs attention for its
512 query tokens and the MLP on them.

All matmul inputs are bf16 (residual path stays f32); every weight is
loaded once. The host pre-arranges all inputs into the exact on-chip tile
layout ([P, ...] per-partition contiguous rows), split into per-ct
sub-DMAs for DMA-engine parallelism. Phase B1 streams 512-token chunks:
LN1 -> K proj (all 8 head pairs) -> V proj, so lnkv is a rotating 2-buffer
stream. Phase B2 is head-pair-outer attention with AV accumulated across
all 16 kv subchunks in PSUM (ACT-exp-bound at ~96% occupancy, the floor
for this sharding); the two K=64 score matmuls of a head pair go to PE
row groups 0/64 and run concurrently (PE row tiling). LayerNorm rstd uses
exp(-0.5*ln(var+eps)) on ACT; mu/rstd and softmax 1/den broadcasts along
partitions use PE ones-outer-products into PSUM instead of slow gpsimd
partition_broadcast; softmax denominators get one batched DVE reciprocal
per head pair, deferred into the next head pair's score loop so the PE
never waits on it.
"""
import numpy as np

import concourse.bass as bass
import concourse.mybir as mybir
import concourse.tile as tile
from concourse import bacc
from concourse.bass_utils import run_bass_kernel_spmd

P = 128
C = 1024
NCT = C // P          # 8 feature tiles
TKV = 2048            # kv tokens per core (sequence length)
TQ = 512              # query tokens per core
HID = 4096
NHT = HID // P        # 32 hidden tiles
H = 16
HD = 64
NHP = H // 2          # 8 head pairs
NJL = TKV // P        # 16 kv subchunks of 128
CHUNK = 512           # ln1/K/V processing chunk
NCHUNK = TKV // CHUNK # 4
EPS = 1e-5
SCALE = HD ** -0.5

f32 = mybir.dt.float32
f32r = mybir.dt.float32r
bf16 = mybir.dt.bfloat16
Act = mybir.ActivationFunctionType


def build_program(sim_standin=False):
    # CoreSim lacks Gelu; Tanh has identical ACT cost, so the sim variant
    # swaps it in for modeled-time runs.
    gelu_fn = Act.Tanh if sim_standin else Act.Gelu
    nc = bacc.Bacc()

    # DRAM I/O (per core). Host pre-arranges everything into the exact
    # on-chip tile layout, so every DMA is per-partition contiguous
    # (128 descriptors instead of 1024+).
    xkvA = nc.dram_tensor("xkvA", [P, NCHUNK, NCT, CHUNK], bf16,
                          kind="ExternalInput")
    xqbA = nc.dram_tensor("xqbA", [P, NCT, TQ], bf16, kind="ExternalInput")
    xqA = nc.dram_tensor("xqA", [P, NCT, TQ], f32r, kind="ExternalInput")
    wqA = nc.dram_tensor("wqA", [P, NHP, NCT, P], bf16,
                         kind="ExternalInput")
    wkA = nc.dram_tensor("wkA", [P, NHP, NCT, P], bf16,
                         kind="ExternalInput")
    wvA = nc.dram_tensor("wvA", [P, NCT, C], bf16, kind="ExternalInput")
    wpA = nc.dram_tensor("wpA", [P, NCT, NHP, P], bf16,
                         kind="ExternalInput")
    w1A = nc.dram_tensor("w1A", [P, NHT // 4, NCT, 512], bf16,
                         kind="ExternalInput")
    w2A = nc.dram_tensor("w2A", [P, NHT // 4, 2, 4, 512], bf16,
                         kind="ExternalInput")
    # all per-feature params packed host-side into one [P, 80] array:
    # cols 0:8 ln1_g, 8:16 ln1_b, 16:24 ln2_g, 24:32 ln2_b, 32:40 b_proj,
    # 40:72 b_fc1, 72:80 b_fc2
    pparams = nc.dram_tensor("pparams", [P, 80], f32, kind="ExternalInput")
    outT = nc.dram_tensor("outT", [C, TQ], f32, kind="ExternalOutput")

    with tile.TileContext(nc) as tc:
      with (
          tc.tile_pool(name="const", bufs=1) as const,
          tc.tile_pool(name="outer", bufs=1) as outer,
      ):
        onesf_row = const.tile([P, 256], f32)
        nc.vector.memset(onesf_row[:], 1.0)
        ones_row = const.tile([P, 256], bf16)
        nc.vector.tensor_copy(ones_row[:], onesf_row[:])
        ones_bf = const.tile([P, 1], bf16)
        nc.vector.tensor_copy(ones_bf[:], onesf_row[:, 0:1])
        ones32 = const.tile([P, 1], f32r)
        nc.vector.tensor_copy(ones32[:], onesf_row[:, 0:1])
        ones32_mm = const.tile([P, P], f32r)
        nc.vector.tensor_copy(ones32_mm[:], onesf_row[:, 0:P])
        eps_t = const.tile([P, 1], f32)
        nc.vector.memset(eps_t[:], EPS)

        pp_t = const.tile([P, 80], f32)
        nc.sync.dma_start(pp_t[:], pparams[:])

        x2T = outer.tile([P, NCT, TQ], f32r)  # attn residual output
        mrs2 = outer.tile([P, 2 * TQ], f32)   # LN2 mu | rstd, broadcast

        def ln_stats(ps_pool, sb_pool, src_ap, F):
            """Stage 1: load x chunk, emit squares + stats matmuls."""
            xt = sb_pool.tile([P, NCT, F], bf16, tag="xkv", bufs=3,
                              name="x_t")
            for ct in range(NCT):
                nc.sync.dma_start(xt[:, ct, :], src_ap[:, ct])
            ps_stat = ps_pool.tile([P, 2 * TQ], f32, tag="big", bufs=2,
                                   name="ps_stat")
            for ct in range(NCT):
                sq = sb_pool.tile([P, F], bf16, tag="ln_sq", bufs=3)
                nc.scalar.activation(sq[:], xt[:, ct, :], Act.Square)
                nc.tensor.matmul(ps_stat[0:1, 0:F], ones_bf[:], xt[:, ct, :],
                                 start=(ct == 0), stop=(ct == NCT - 1))
                nc.tensor.matmul(ps_stat[0:1, TQ:TQ + F], ones_bf[:], sq[:],
                                 start=(ct == 0), stop=(ct == NCT - 1))
            return xt, ps_stat

        def ln_smalls(sb_pool, ps_stat, F):
            """Stage 2: mu/var tiny ops + rstd via ACT ln/exp. Emitted
            ahead of the previous chunk's normalize so these 1-lane ops
            don't queue behind 24 big DVE ops."""
            mu = sb_pool.tile([1, F], f32r, tag="ln_mu", bufs=3)
            var = sb_pool.tile([1, F], f32r, tag="ln_var", bufs=3)
            nc.vector.tensor_scalar_mul(mu[:], ps_stat[0:1, 0:F], 1.0 / C)
            nc.vector.tensor_scalar_mul(var[:], ps_stat[0:1, TQ:TQ + F],
                                        1.0 / C)
            mu2 = sb_pool.tile([1, F], f32r, tag="ln_mu2", bufs=3)
            nc.vector.tensor_mul(mu2[:], mu[:], mu[:])
            nc.vector.tensor_sub(var[:], var[:], mu2[:])
            # rstd = exp(-0.5 * ln(var + eps))
            nc.scalar.activation(var[:], var[:], Act.Ln, bias=eps_t[0:1, :])
            nc.scalar.activation(var[:], var[:], Act.Exp, scale=-0.5)
            return mu, var

        def ln_apply(ps_pool, sb_pool, xt, mu, var, F, dst_name):
            """Stage 3: PE ones-product broadcast of mu/rstd, normalize."""
            ps_bc = ps_pool.tile([P, 2 * TQ], f32, tag="bc", bufs=1,
                                 name="ps_bc")
            nc.tensor.matmul(ps_bc[:, 0:F], ones32_mm[0:1, :], mu[:],
                             start=True, stop=True)
            nc.tensor.matmul(ps_bc[:, TQ:TQ + F], ones32_mm[0:1, :], var[:],
                             start=True, stop=True)
            mrs = sb_pool.tile([P, 2 * TQ], bf16, tag="ln_mrs", bufs=2,
                               name="mrs")
            nc.vector.tensor_copy(mrs[:], ps_bc[:])
            ln = sb_pool.tile([P, NCT, F], bf16, tag="lnkv", bufs=2,
                              name=dst_name)
            for ct in range(NCT):
                o = ln[:, ct, :]
                tmp = sb_pool.tile([P, F], bf16, tag="ln_tmp", bufs=3)
                nc.vector.tensor_sub(tmp[:], xt[:, ct, :], mrs[:, 0:F])
                nc.vector.tensor_mul(tmp[:], tmp[:], mrs[:, TQ:TQ + F])
                nc.vector.tensor_scalar(o, tmp[:], pp_t[:, 0 + ct:1 + ct],
                                        pp_t[:, 8 + ct:9 + ct],
                                        op0=mybir.AluOpType.mult,
                                        op1=mybir.AluOpType.add)
            return ln

        with tc.tile_pool(name="pa_big", bufs=1) as pa_big:
            kT = pa_big.tile([P, NHP, TKV], bf16)      # K, pair-dim major
            v_c = pa_big.tile([P, NJL, H, HD + 1], bf16)
            qT = pa_big.tile([P, NHP, TQ], bf16)
            attnT = pa_big.tile([P, NHP, TQ], bf16)    # normalized AV
            den = pa_big.tile([P, TQ], f32r)  # head A -> p0, head B -> p64
            rden = pa_big.tile([P, TQ], f32r)
            nc.vector.tensor_copy(den[:, 0:256], onesf_row[:])
            nc.vector.tensor_copy(den[:, 256:TQ], onesf_row[:])

            # ones row for denominators
            nc.vector.tensor_copy(
                v_c[:, :, :, HD],
                ones_row[:].rearrange("p (a b) -> p a b", a=NJL))

            # ---- Phase B1: LN1 + Q, then per chunk LN1 -> K -> V ----
            with (
                tc.tile_pool(name="pb1", bufs=1) as pb1,
                tc.tile_pool(name="ps1", bufs=1, space="PSUM") as pa_ps,
            ):
                wv_t = pb1.tile([P, NCT, C], bf16, name="wv_t")
                # issue on the ACT DMA queue so it doesn't serialize behind
                # the startup x-chunk loads on the sync queue
                for ct in range(NCT):
                    nc.scalar.dma_start(wv_t[:, ct, :], wvA[:, ct])

                def emit_k(lnkv, j0):
                    for hp in range(NHP):
                        wk_t = pb1.tile([P, NCT, P], bf16, tag="wk", bufs=2,
                                        name="wk_t")
                        nc.sync.dma_start(wk_t[:], wkA[:, hp])
                        ps = pa_ps.tile([P, CHUNK], f32, tag="kvq", bufs=2,
                                        name="k_ps")
                        for ct in range(NCT):
                            nc.tensor.matmul(ps[:], wk_t[:, ct, :],
                                             lnkv[:, ct, :],
                                             start=(ct == 0),
                                             stop=(ct == NCT - 1))
                        if hp % 2 == 0:
                            nc.scalar.copy(kT[:, hp, j0:j0 + CHUNK], ps[:])
                        else:
                            nc.vector.tensor_copy(kT[:, hp, j0:j0 + CHUNK],
                                                  ps[:])

                def emit_v(lnkv, ch):
                    for half in range(2):
                        for jloc in range(4):
                            jl = ch * 4 + jloc
                            ps = pa_ps.tile([P, CHUNK], f32, tag="kvq",
                                            bufs=2, name="v_ps")
                            for ct in range(NCT):
                                nc.tensor.matmul(
                                    ps[:],
                                    lnkv[:, ct, jloc * P:(jloc + 1) * P],
                                    wv_t[:, ct,
                                         half * CHUNK:(half + 1) * CHUNK],
                                    start=(ct == 0), stop=(ct == NCT - 1))
                            nc.vector.tensor_copy(
                                v_c[:, jl, half * 8:(half + 1) * 8, 0:HD],
                                ps[:].rearrange("p (h d) -> p h d", d=HD))

                def ln_chunk(src_ap, F, dst_name):
                    xt, ps_stat = ln_stats(pa_ps, pb1, src_ap, F)
                    mu, var = ln_smalls(pb1, ps_stat, F)
                    return ln_apply(pa_ps, pb1, xt, mu, var, F, dst_name)

                def emit_q():
                    # LN1 of the query block -> Q projection (emitted after
                    # chunk 1 so its serial LN chain hides under K/V work)
                    ln1q = ln_chunk(xqbA[:], TQ, "ln1q")
                    for hp in range(NHP):
                        wq_t = pb1.tile([P, NCT, P], bf16, tag="wk", bufs=2,
                                        name="wq_t")
                        nc.sync.dma_start(wq_t[:], wqA[:, hp])
                        ps = pa_ps.tile([P, TQ], f32, tag="kvq", bufs=2,
                                        name="q_ps")
                        for ct in range(NCT):
                            nc.tensor.matmul(ps[:], wq_t[:, ct, :],
                                             ln1q[:, ct, :],
                                             start=(ct == 0),
                                             stop=(ct == NCT - 1))
                        nc.scalar.copy(qT[:, hp, :], ps[:])

                for ch in range(NCHUNK):
                    lnkv = ln_chunk(xkvA[:, ch], CHUNK, "lnkv")
                    emit_k(lnkv, ch * CHUNK)
                    emit_v(lnkv, ch)
                    if ch == 1:
                        emit_q()

            # ---- Phase B2: attention (hp-outer), then proj + LN2 ----
            with (
                tc.tile_pool(name="pb2", bufs=1) as pb2,
                tc.tile_pool(name="ps2", bufs=1, space="PSUM") as pa_ps,
            ):
                xq = pb2.tile([P, NCT, TQ], f32r, name="xq")
                for ct in range(NCT):
                    nc.sync.dma_start(xq[:, ct, :], xqA[:, ct])
                wp_t = pb2.tile([P, NCT, NHP, P], bf16, name="wp_t")
                for ct in range(NCT):
                    nc.sync.dma_start(wp_t[:, ct], wpA[:, ct])

                def normalize(hp, ps_av0, ps_av1):
                    """Broadcast 1/den along partitions via a PE
                    ones-outer-product, then scale AV on DVE."""
                    ps_rcp = pa_ps.tile([P, 2 * TQ], f32, tag="big",
                                        bufs=2, name="ps_rcp")
                    nc.tensor.matmul(ps_rcp[:, 0:TQ], ones32_mm[0:1, :],
                                     rden[0:1, :], start=True, stop=True)
                    nc.tensor.matmul(ps_rcp[:, TQ:2 * TQ],
                                     ones32_mm[HD:HD + 1, :],
                                     rden[HD:HD + 1, :],
                                     start=True, stop=True)
                    rcp_sb = pb2.tile([P, 2 * TQ], f32, tag="rcpb", bufs=2,
                                      name="rcp_sb")
                    nc.vector.tensor_copy(rcp_sb[:], ps_rcp[:])
                    for i, ps_av in ((0, ps_av0), (1, ps_av1)):
                        nc.vector.tensor_mul(
                            attnT[i * HD:(i + 1) * HD, hp, :],
                            ps_av[0:HD, :],
                            rcp_sb[0:HD, i * TQ:(i + 1) * TQ])

                pend = None
                for hp in range(NHP):
                    ps_av0 = pa_ps.tile([HD + 1, TQ], f32, tag="av0",
                                        bufs=2, name="ps_av0")
                    ps_av1 = pa_ps.tile([HD + 1, TQ], f32, tag="av1",
                                        bufs=2, name="ps_av1")
                    for jl in range(NJL):
                        ps_sc = pa_ps.tile([P, 2 * TQ], f32, tag="big",
                                           bufs=2, name="ps_sc")
                        nc.tensor.matmul(
                            ps_sc[:, 0:TQ],
                            kT[0:HD, hp, jl * P:(jl + 1) * P],
                            qT[0:HD, hp, :], start=True, stop=True)
                        nc.tensor.matmul(
                            ps_sc[:, TQ:2 * TQ],
                            kT[HD:P, hp, jl * P:(jl + 1) * P],
                            qT[HD:P, hp, :], start=True, stop=True)
                        e_sb = pb2.tile([P, 2 * TQ], bf16, tag="e", bufs=3,
                                        name="e_sb")
                        nc.scalar.activation(e_sb[:], ps_sc[:], Act.Exp,
                                             scale=SCALE)
                        nc.tensor.matmul(
                            ps_av0[:], v_c[:, jl, 2 * hp, :], e_sb[:, 0:TQ],
                            start=(jl == 0), stop=(jl == NJL - 1))
                        nc.tensor.matmul(
                            ps_av1[:], v_c[:, jl, 2 * hp + 1, :],
                            e_sb[:, TQ:2 * TQ],
                            start=(jl == 0), stop=(jl == NJL - 1))
                        if jl == 4 and pend is not None:
                            normalize(*pend)
                            pend = None
                    # denominators: head A -> partition 0, head B -> 64
                    nc.vector.tensor_copy(den[0:1, :], ps_av0[HD:HD + 1, :])
                    nc.vector.tensor_copy(den[HD:HD + 1, :],
                                          ps_av1[HD:HD + 1, :])
                    with nc.allow_low_precision(
                            reason="f32r is bit-identical to f32"):
                        nc.vector.reciprocal(rden[:], den[:])
                    pend = (hp, ps_av0, ps_av1)
                normalize(*pend)

                # proj + residual + LN2 stats (stat accumulators reuse the
                # av banks, which are free once hp7 is normalized)
                ps_st2a = pa_ps.tile([HD + 1, TQ], f32, tag="av0", bufs=2,
                                     name="ps_st2a")
                ps_st2b = pa_ps.tile([HD + 1, TQ], f32, tag="av1", bufs=2,
                                     name="ps_st2b")
                for ct in range(NCT):
                    ps_b = pa_ps.tile([P, 2 * TQ], f32, tag="big", bufs=2,
                                      name="proj_ps")
                    ps = ps_b[:, 0:TQ]
                    for hp in range(NHP):
                        nc.tensor.matmul(ps[:], wp_t[:, ct, hp, :],
                                         attnT[:, hp, :],
                                         start=(hp == 0),
                                         stop=(hp == NHP - 1))
                    o = x2T[:, ct, :]
                    nc.scalar.activation(o, ps[:], Act.Identity,
                                         bias=pp_t[:, 32 + ct:33 + ct])
                    nc.vector.tensor_add(o, o, xq[:, ct, :])
                    sq2 = pb2.tile([P, TQ], bf16, tag="sq2", bufs=3,
                                   name="sq2")
                    nc.scalar.activation(sq2[:], o, Act.Square)
                    nc.tensor.matmul(ps_st2a[0:1, :], ones32[:], o,
                                     start=(ct == 0), stop=(ct == NCT - 1))
                    nc.tensor.matmul(ps_st2b[0:1, :], ones_bf[:], sq2[:],
                                     start=(ct == 0), stop=(ct == NCT - 1))

                mu = pb2.tile([1, TQ], f32r, tag="ln_mu", bufs=1, name="mu2")
                var = pb2.tile([1, TQ], f32r, tag="ln_var", bufs=1,
                               name="var2")
                nc.vector.tensor_scalar_mul(mu[:], ps_st2a[0:1, :], 1.0 / C)
                nc.vector.tensor_scalar_mul(var[:], ps_st2b[0:1, :], 1.0 / C)
                mu2_t = pb2.tile([1, TQ], f32r, tag="ln_mu2", bufs=1,
                                 name="mu2sq")
                nc.vector.tensor_mul(mu2_t[:], mu[:], mu[:])
                nc.vector.tensor_sub(var[:], var[:], mu2_t[:])
                nc.scalar.activation(var[:], var[:], Act.Ln,
                                     bias=eps_t[0:1, :])
                nc.scalar.activation(var[:], var[:], Act.Exp, scale=-0.5)
                ps_bc2 = pa_ps.tile([P, 2 * TQ], f32, tag="big", bufs=2,
                                    name="ps_bc2")
                nc.tensor.matmul(ps_bc2[:, 0:TQ], ones32_mm[0:1, :], mu[:],
                                 start=True, stop=True)
                nc.tensor.matmul(ps_bc2[:, TQ:2 * TQ], ones32_mm[0:1, :],
                                 var[:], start=True, stop=True)
                nc.vector.tensor_copy(mrs2[:], ps_bc2[:])

        # ---- Phase D: LN2 normalize, fc1+gelu, fc2 + residual ----
        with (
            tc.tile_pool(name="pd_sb", bufs=1) as pd_sb,
            tc.tile_pool(name="pd_g", bufs=1) as pd_g,
            tc.tile_pool(name="pd_w", bufs=1) as pd_w,
            tc.tile_pool(name="pd_ps", bufs=1, space="PSUM") as pd_ps,
            tc.tile_pool(name="pd_ps2", bufs=1, space="PSUM") as pd_ps2,
        ):
            ln2T = pd_g.tile([P, NCT, TQ], bf16, name="ln2T")
            for ct in range(NCT):
                o = ln2T[:, ct, :]
                tmp = pd_sb.tile([P, TQ], f32, tag=f"l2t{ct % 2}", bufs=2,
                                 name="l2tmp")
                nc.vector.tensor_sub(tmp[:], x2T[:, ct, :], mrs2[:, 0:TQ])
                nc.vector.tensor_mul(tmp[:], tmp[:], mrs2[:, TQ:2 * TQ])
                nc.vector.tensor_scalar(o, tmp[:], pp_t[:, 16 + ct:17 + ct],
                                        pp_t[:, 24 + ct:25 + ct],
                                        op0=mybir.AluOpType.mult,
                                        op1=mybir.AluOpType.add)

            g1T = pd_g.tile([P, NHT, TQ], bf16, name="g1T")
            for htg in range(NHT // 4):
                w1_t = pd_w.tile([P, NCT, 512], bf16, tag="w1", bufs=2,
                                 name="w1_t")
                for ct2 in range(0, NCT, 2):
                    nc.sync.dma_start(w1_t[:, ct2:ct2 + 2, :],
                                      w1A[:, htg, ct2:ct2 + 2])
                for hl in range(4):
                    ht = htg * 4 + hl
                    ps = pd_ps.tile([P, TQ], f32, tag="fc1_ps", bufs=2,
                                    name="fc1_ps")
                    for ct in range(NCT):
                        nc.tensor.matmul(
                            ps[:], w1_t[:, ct, hl * P:(hl + 1) * P],
                            ln2T[:, ct, :],
                            start=(ct == 0), stop=(ct == NCT - 1))
                    nc.scalar.activation(g1T[:, ht, :], ps[:], gelu_fn,
                                         bias=pp_t[:, 40 + ht:41 + ht])

            for ctg in range(2):
                ps_out = [pd_ps2.tile([P, TQ], f32, tag=f"fc2_{i}",
                                      name=f"fc2_ps_{i}")
                          for i in range(4)]
                for htg4 in range(NHT // 4):
                    w2_t = pd_w.tile([P, 4, 512], bf16, tag="w2", bufs=3,
                                     name="w2_t")
                    for h2 in range(0, 4, 2):
                        nc.sync.dma_start(w2_t[:, h2:h2 + 2, :],
                                          w2A[:, htg4, ctg, h2:h2 + 2])
                    for hl in range(4):
                        ht = htg4 * 4 + hl
                        for cl in range(4):
                            nc.tensor.matmul(
                                ps_out[cl][:],
                                w2_t[:, hl, cl * P:(cl + 1) * P],
                                g1T[:, ht, :],
                                start=(ht == 0), stop=(ht == NHT - 1))
                for cl in range(4):
                    ct = ctg * 4 + cl
                    o = pd_sb.tile([P, TQ], f32, tag="out_t", bufs=3,
                                   name="out_t")
                    nc.scalar.activation(o[:], ps_out[cl][:], Act.Identity,
                                         bias=pp_t[:, 72 + ct:73 + ct])
                    nc.vector.tensor_add(o[:], o[:], x2T[:, ct, :])
                    nc.sync.dma_start(outT[ct * P:(ct + 1) * P, :], o[:])

    nc.finalize()
    return nc


_program = None

# test.py can set RUN_OPTS (e.g. trace=True) and read LAST_RESULTS for
# exec_time_ns / trace paths. The grading harness uses neither.
RUN_OPTS = {}
LAST_RESULTS = None


def _get_program():
    global _program
    if _program is None:
        _program = build_program()
    return _program


def pack_params(inputs):
    """Pack per-feature params into [P, 80] (see pparams layout comment)."""
    def cols(name, n):
        return np.asarray(inputs[name], np.float32).reshape(n // P, P).T
    return np.ascontiguousarray(np.concatenate([
        cols("ln1_g", C), cols("ln1_b", C), cols("ln2_g", C),
        cols("ln2_b", C), cols("b_proj", C), cols("b_fc1", HID),
        cols("b_fc2", C)], axis=1))


def make_in_maps(inputs):
    import ml_dtypes

    bf = ml_dtypes.bfloat16
    x = np.asarray(inputs["x"], dtype=np.float32)
    B, N, _ = x.shape  # [2, 2048, 1024]

    w_qkv = np.asarray(inputs["w_qkv"], dtype=np.float32)

    def arr(a):
        return np.ascontiguousarray(a)

    # weights pre-arranged into on-chip tile layouts (see dram_tensor
    # comments); all are [P, ...] with contiguous per-partition rows
    wqT = w_qkv[0:C].T
    wkT = w_qkv[C:2 * C].T
    wvT = w_qkv[2 * C:3 * C].T
    wpT = np.asarray(inputs["w_proj"], np.float32).T
    w1T = np.asarray(inputs["w_fc1"], np.float32).T
    w2T = np.asarray(inputs["w_fc2"], np.float32).T
    shared = {
        "wqA": arr(wqT.reshape(8, P, 8, P).transpose(1, 2, 0, 3).astype(bf)),
        "wkA": arr(wkT.reshape(8, P, 8, P).transpose(1, 2, 0, 3).astype(bf)),
        "wvA": arr(wvT.reshape(8, P, C).transpose(1, 0, 2).astype(bf)),
        "wpA": arr(wpT.reshape(8, P, 8, P).transpose(1, 2, 0, 3).astype(bf)),
        "w1A": arr(w1T.reshape(8, P, 8, 512).transpose(1, 2, 0, 3)
                   .astype(bf)),
        "w2A": arr(w2T.reshape(8, 4, P, 2, 512).transpose(2, 0, 3, 1, 4)
                   .astype(bf)),
        "pparams": pack_params(inputs),
    }
    in_maps = []
    for core in range(8):
        b, qb = core // 4, core % 4
        xT = x[b].T  # [C, N] f32
        xq = xT[:, qb * TQ:(qb + 1) * TQ]
        m = dict(shared)
        m["xkvA"] = arr(xT.reshape(8, P, 4, 512).transpose(1, 2, 0, 3)
                        .astype(bf))
        m["xqbA"] = arr(xq.reshape(8, P, TQ).transpose(1, 0, 2).astype(bf))
        m["xqA"] = arr(xq.reshape(8, P, TQ).transpose(1, 0, 2)
                       .astype(np.float32))
        in_maps.append(m)
    return in_maps


def kernel(**inputs):
    in_maps = make_in_maps(inputs)
    x = np.asarray(inputs["x"], dtype=np.float32)
    B, N, _ = x.shape

    nc = _get_program()
    global LAST_RESULTS
    res = run_bass_kernel_spmd(nc, in_maps, list(range(8)), **RUN_OPTS)
    LAST_RESULTS = res

    out = np.empty((B, N, C), dtype=np.float32)
    for core in range(8):
        b, qb = core // 4, core % 4
        out[b, qb * TQ:(qb + 1) * TQ, :] = res.results[core]["outT"].T
    return out
